# revision 1
# baseline (speedup 1.0000x reference)
"""AnyPrecisionLinear (4-bit LUT dequant + CSR outliers + bias) on 8 TRN2 cores.

Sharding: 4-way over out_features (O) x 2-way over tokens (B*S).
Core c handles o in [1024*(c%4), +1024), tokens [4096*(c//4), +4096).

Device does all value math:
  - W (bf16) built from lut via one broadcast-copy + GPSIMD local_scatter with
    host-computed slot tables (pure index preprocessing of qweight bits).
  - CSR outlier values converted and scattered on device, added to W.
  - x converted f32->bf16 on ScalarE, transposed via DMA xbar.
  - GEMM on TensorE (bf16, f32 PSUM accum), bias folded in as a K=1 matmul.
Host does only layout/index work: sharding, bit-plane->index repack, sort/slot
tables, CSR indptr parsing + dedup, output concat.
"""

import os
import numpy as np
from contextlib import ExitStack

# Problem constants (hardcoded per harness contract).
B, S, I, O = 4, 2048, 4096, 4096
W_BITS = 4
NT_TOTAL = B * S          # 8192 tokens
N_CORES = 8
O_WAY, N_WAY = 4, 2       # sharding grid
O_SH = O // O_WAY         # 1024 out features per core
N_SH = NT_TOTAL // N_WAY  # 4096 tokens per core
NCHUNK = N_SH // 128      # 32 token chunks per core
OT = O_SH // 128          # 8 o-tiles of 128 rows per core
CH = 1024                 # i-chunk size for local_scatter (num_elems limit 2046)
NCH = I // CH             # 4
IC = I // 128             # 32 i-blocks for the GEMM

_GRAPH_CACHE = {}
PE_FRAC = 0.5
RING_MODE = "single"

import ml_dtypes

_EYE = np.eye(128, dtype=ml_dtypes.bfloat16)


def _host_indices(qweight):
    """bit-planes -> 4-bit index array [O, I] (uint8). Pure bit relayout."""
    shifts = np.arange(32, dtype=np.int32)
    # bits[b, o, w, s] = bit s of qweight[b, o, w]
    bits = ((qweight[:, :, :, None] >> shifts) & 1).astype(np.uint8)
    planew = (1 << (W_BITS - 1 - np.arange(W_BITS))).astype(np.uint8)
    idx = (bits * planew[:, None, None, None]).sum(axis=0, dtype=np.int32)
    return idx.reshape(O, I).astype(np.uint8)


def _scatter_tables(idx, rows, cols, vals):
    """Merged dequant+CSR local_scatter tables.

    Device scatters DELTA values (lut[v]-lut[0]; csr positions get
    cv+lut[v]-lut[0]) and then adds lut[0] per partition, so unwritten
    (value-0, non-csr) positions come out right with no predication.

    Table layout per (row o, chunk ch): [CSE csr slots | 16*S dequant slots];
    dequant slot 16*s+v = position of s-th occurrence of v (v>=1, csr
    positions excluded).  Returns:
      tbl   [O, NCH, CSE+NI] int16   scatter indices (-1 pad)
      vsel  [O, NCH, CSE]    int16   lut index of each csr slot (0 pad)
      cvals [O, NCH, CSE]    f32     csr value of each slot (0 pad)
      NI, CSE
    """
    # ---- CSR -> dedup'd COO ----
    nnz = cols.shape[0]
    row_ids = (np.searchsorted(rows, np.arange(nnz), side="right") - 1).astype(np.int64)
    key = row_ids * I + cols.astype(np.int64)
    uk, inv = np.unique(key, return_inverse=True)
    v2 = np.zeros(len(uk), np.float64)
    np.add.at(v2, inv, vals.astype(np.float64))
    r2 = uk // I
    c2 = uk % I
    ch2 = c2 // CH
    cl2 = (c2 % CH).astype(np.int16)
    grp = r2 * NCH + ch2  # ascending (uk sorted)
    _, gstart, gcount = np.unique(grp, return_index=True, return_counts=True)
    CSE = int(gcount.max())
    CSE += CSE % 2
    CSE = max(CSE, 2)
    rank = np.arange(len(uk)) - np.repeat(gstart, gcount)

    is_csr = np.zeros((O, NCH, CH), bool)
    is_csr[r2, ch2, cl2] = True

    # ---- dequant slots (v>=1, non-csr) ----
    idx4 = idx.reshape(O, NCH, CH).astype(np.int16)
    order = np.argsort(idx4, axis=-1, kind="stable").astype(np.int16)
    sortedv = np.take_along_axis(idx4, order.astype(np.int64), axis=-1)
    cnt = np.zeros((O, NCH, 16), np.int32)
    for v in range(16):
        cnt[:, :, v] = (idx4 == v).sum(-1)
    S = int(cnt[:, :, 1:].max())
    NI = 16 * S
    if NI % 2:
        NI += 16
    cstart = np.concatenate(
        [np.zeros((O, NCH, 1), np.int32), np.cumsum(cnt, -1)[:, :, :-1]], -1
    )
    srank = np.arange(CH)[None, None, :] - np.take_along_axis(
        cstart, sortedv.astype(np.int64), axis=-1
    )
    sorted_is_csr = np.take_along_axis(is_csr, order.astype(np.int64), axis=-1)
    keep = (sortedv > 0) & ~sorted_is_csr
    W = CSE + NI
    scratch = NI + 16 * (CH // 16 + 2)
    tbl = np.full((O, NCH, CSE + scratch), -1, np.int16)
    flat = (CSE + 16 * srank + sortedv).astype(np.int64)
    np.put_along_axis(
        tbl, np.where(keep, flat, tbl.shape[-1] - 1),
        np.where(keep, order, -1), axis=-1,
    )
    tbl = tbl[:, :, :W].copy()
    # ---- csr slots ----
    vsel = np.zeros((O, NCH, CSE), np.int16)
    cvals = np.zeros((O, NCH, CSE), np.float32)
    csr_tbl = np.full((O, NCH, CSE), -1, np.int16)
    csr_tbl[r2, ch2, rank] = cl2
    vsel[r2, ch2, rank] = idx4[r2, ch2, cl2.astype(np.int64)]
    cvals[r2, ch2, rank] = v2.astype(np.float32)
    tbl[:, :, :CSE] = csr_tbl
    return tbl, vsel, cvals, NI, CSE


def _build_graph(NI, CS, pe_frac=0.5, parts="dwxg", ring_mode="split"):
    # parts: "d" dequant-scatter, "w" W-transposes, "x" x pipeline, "g" GEMM+y
    import concourse.bass as bass
    import concourse.bacc as bacc
    import concourse.tile as tile
    from concourse import mybir

    dt = mybir.dt
    nc = bacc.Bacc("TRN2", target_bir_lowering=False, debug=False)

    CSE = CS
    W = CSE + NI
    x_d = nc.dram_tensor("x", [NCHUNK, 128, I], dt.float32, kind="ExternalInput")
    lut_d = nc.dram_tensor("lut", [OT, 128, 16], dt.float32, kind="ExternalInput")
    qid_d = nc.dram_tensor("qidx", [OT, 128, NCH * W], dt.int16, kind="ExternalInput")
    vse_d = nc.dram_tensor("vsel", [OT, 128, NCH * CSE], dt.int16, kind="ExternalInput")
    cva_d = nc.dram_tensor("cvals", [OT, 128, NCH * CSE], dt.float32, kind="ExternalInput")
    bias_d = nc.dram_tensor("bias", [1, O_SH], dt.float32, kind="ExternalInput")
    eye_d = nc.dram_tensor("eye", [128, 128], dt.bfloat16, kind="ExternalInput")
    y_d = nc.dram_tensor("y", [NCHUNK, 128, O_SH], dt.float32, kind="ExternalOutput")
    # ic-blocks transposed on the PE (rest go through the DMA xbar rings)
    pe_ics = set(range(IC))
    n_pe = int(IC * pe_frac)
    pe_ics = set(range(0, IC, max(1, IC // max(n_pe, 1))))  # spread evenly
    while len(pe_ics) > n_pe:
        pe_ics.pop()

    ld_eng = nc.sync
    big_eng = nc.sync
    tr_eng = nc.sync if ring_mode == "single" else nc.scalar
    with tile.TileContext(nc) as tc, ExitStack() as ctx:
        const = ctx.enter_context(tc.tile_pool(name="const", bufs=1))
        wpool = ctx.enter_context(tc.tile_pool(name="w", bufs=2))
        spool = ctx.enter_context(tc.tile_pool(name="scat", bufs=2))
        qpool = ctx.enter_context(tc.tile_pool(name="qp", bufs=1))
        xfpool = ctx.enter_context(tc.tile_pool(name="xf", bufs=2))
        xpool = ctx.enter_context(tc.tile_pool(name="x", bufs=2))
        yopool = ctx.enter_context(tc.tile_pool(name="yo", bufs=2))
        psum = ctx.enter_context(
            tc.tile_pool(name="ps", bufs=2, space=bass.MemorySpace.PSUM)
        )
        pst = ctx.enter_context(
            tc.tile_pool(name="pst", bufs=4, space=bass.MemorySpace.PSUM)
        )

        # Resident transposed weights: WT[p, 1024*ic + ol] = W[ol, 128*ic + p]
        WT = const.tile([128, IC * O_SH], dt.bfloat16)

        ones = const.tile([1, 128], dt.bfloat16)
        nc.vector.memset(ones[:, :], 1.0)
        browf = const.tile([1, O_SH], dt.float32)
        nc.sync.dma_start(browf[:, :], bias_d[:, :])
        brow = const.tile([1, O_SH], dt.bfloat16)
        nc.scalar.copy(brow[:, :], browf[:, :])
        eye = const.tile([128, 128], dt.bfloat16)
        ld_eng.dma_start(eye[:, :], eye_d[:, :])

        if "d" not in parts:
            nc.vector.memset(WT[:, 0:512], 0.125)
        # ---- dequant + CSR (merged single scatter per chunk) ----
        for t in range(OT if "d" in parts else 0):
            lutf = spool.tile([128, 16], dt.float32, tag="lutf")
            ld_eng.dma_start(lutf[:, :], lut_d[t])
            lutb = spool.tile([128, 16], dt.bfloat16, tag="lutb")
            nc.vector.tensor_copy(lutb[:, :], lutf[:, :])
            lutdf = spool.tile([128, 16], dt.float32, tag="lutdf")
            nc.vector.tensor_scalar(
                lutdf[:, :], lutf[:, :], lutf[:, 0:1], None,
                mybir.AluOpType.subtract,
            )
            lutd = spool.tile([128, 16], dt.bfloat16, tag="lutd")
            nc.vector.tensor_copy(lutd[:, :], lutdf[:, :])
            # delta-lut pattern repeated SLOTS times (log-doubling copies)
            pat = spool.tile([128, NI], dt.bfloat16, tag="pat")
            nc.vector.tensor_copy(pat[:, 0:16], lutd[:, :])
            sz = 16
            while sz < NI:
                cp = min(sz, NI - sz)
                nc.vector.tensor_copy(pat[:, sz : sz + cp], pat[:, 0:cp])
                sz += cp
            # csr combined deltas: cv + lutd[v]
            vsl = spool.tile([128, NCH * CSE], dt.int16, tag="vsl")
            ld_eng.dma_start(vsl[:, :], vse_d[t])
            cvf = spool.tile([128, NCH * CSE], dt.float32, tag="cvf")
            ld_eng.dma_start(cvf[:, :], cva_d[t])
            comb = spool.tile([128, NCH * CSE], dt.bfloat16, tag="comb")
            nc.vector.tensor_copy(comb[:, :], cvf[:, :])
            tmp = spool.tile([128, NCH * CSE], dt.bfloat16, tag="tmp")
            for v in range(1, 16):
                nc.vector.tensor_scalar(
                    tmp[:, :], vsl[:, :], float(v), lutdf[:, v : v + 1],
                    mybir.AluOpType.is_equal, mybir.AluOpType.mult,
                )
                nc.vector.tensor_add(comb[:, :], comb[:, :], tmp[:, :])
            qix = qpool.tile([128, NCH * W], dt.int16, tag="qix")
            ld_eng.dma_start(qix[:, :], qid_d[t])
            Wt = wpool.tile([128, I], dt.bfloat16, tag="W")
            for ch in range(NCH):
                sl = slice(ch * CH, (ch + 1) * CH)
                data = spool.tile([128, W], dt.bfloat16, tag="data")
                nc.vector.tensor_copy(
                    data[:, 0:CSE], comb[:, ch * CSE : (ch + 1) * CSE]
                )
                nc.vector.tensor_copy(data[:, CSE:], pat[:, :])
                nc.gpsimd.local_scatter(
                    Wt[:, sl], data[:, :], qix[:, ch * W : (ch + 1) * W],
                    channels=128, num_elems=CH, num_idxs=W,
                )
                nc.vector.tensor_scalar(
                    Wt[:, sl], Wt[:, sl], lutf[:, 0:1], None,
                    mybir.AluOpType.add,
                )
            if "w" in parts:
                for ic in range(IC):
                    eng = tr_eng
                    eng.dma_start_transpose(
                        WT[:, O_SH * ic + 128 * t : O_SH * ic + 128 * (t + 1)],
                        Wt[:, 128 * ic : 128 * (ic + 1)],
                    )

        # ---- GEMM ----
        for n in range(NCHUNK if ("x" in parts or "g" in parts) else 0):
            xT = xpool.tile([128, I], dt.bfloat16, tag="xT")
            if "x" in parts:
                xf = xfpool.tile([128, I], dt.float32, tag="xf")
                big_eng.dma_start(xf[:, :], x_d[n])
                xb = xpool.tile([128, I], dt.bfloat16, tag="xb")
                nc.scalar.copy(xb[:, :], xf[:, :])
                for ic in range(IC):
                    src = xb[:, 128 * ic : 128 * (ic + 1)]
                    dst = xT[:, 128 * ic : 128 * (ic + 1)]
                    if ic in pe_ics:
                        pt = pst.tile([128, 128], dt.bfloat16, tag="pt")
                        nc.tensor.transpose(pt[:, :], src, eye[:, :])
                        nc.vector.tensor_copy(dst, pt[:, :])
                    else:
                        eng = tr_eng
                        eng.dma_start_transpose(dst, src)
            elif n == 0:
                nc.vector.memset(xT[:, :], 0.25)
            for blk in range(O_SH // 512 if "g" in parts else 0):
                ps = psum.tile([128, 512], dt.float32, tag="ps")
                nc.tensor.matmul(
                    ps[:, :], ones[:, :], brow[:, 512 * blk : 512 * (blk + 1)],
                    start=True, stop=False,
                )
                for ic in range(IC):
                    nc.tensor.matmul(
                        ps[:, :],
                        xT[:, 128 * ic : 128 * (ic + 1)],
                        WT[:, O_SH * ic + 512 * blk : O_SH * ic + 512 * (blk + 1)],
                        start=False, stop=(ic == IC - 1),
                    )
                yo = yopool.tile([128, 512], dt.float32, tag="yo")
                nc.vector.tensor_copy(yo[:, :], ps[:, :])
                big_eng.dma_start(y_d[n][:, 512 * blk : 512 * (blk + 1)], yo[:, :])

    nc.compile()
    return nc


def _prep_inputs(x, qweight, lut, rows, cols, vals, bias):
    x = np.ascontiguousarray(np.asarray(x, dtype=np.float32))
    qweight = np.asarray(qweight, dtype=np.int32)
    lut = np.asarray(lut, dtype=np.float32)
    rows = np.asarray(rows, dtype=np.int64)
    cols = np.asarray(cols, dtype=np.int64)
    vals = np.asarray(vals, dtype=np.float32)
    bias = np.asarray(bias, dtype=np.float32)

    idx = _host_indices(qweight)
    tbl, vsel, cvals, NI, CSE = _scatter_tables(idx, rows, cols, vals)
    W = CSE + NI

    x2 = x.reshape(NT_TOTAL, I)
    in_maps = []
    for c in range(N_CORES):
        oq, nh = c % O_WAY, c // O_WAY
        osl = slice(O_SH * oq, O_SH * (oq + 1))
        nsl = slice(N_SH * nh, N_SH * (nh + 1))
        in_maps.append(
            {
                "x": np.ascontiguousarray(x2[nsl].reshape(NCHUNK, 128, I)),
                "lut": np.ascontiguousarray(lut[osl].reshape(OT, 128, 16)),
                # chunk-major per o-tile row: [OT, 128, NCH*W]
                "qidx": np.ascontiguousarray(
                    tbl[osl].reshape(OT, 128, NCH * W)
                ),
                "vsel": np.ascontiguousarray(
                    vsel[osl].reshape(OT, 128, NCH * CSE)
                ),
                "cvals": np.ascontiguousarray(
                    cvals[osl].reshape(OT, 128, NCH * CSE)
                ),
                "bias": np.ascontiguousarray(bias[osl].reshape(1, O_SH)),
                "eye": _EYE,
            }
        )
    return in_maps, NI, CSE


def _run(inputs, trace=False, trace_kwargs=None):
    from concourse.bass_utils import run_bass_kernel_spmd

    in_maps, NI, CS = _prep_inputs(**inputs)

    key = (NI, CS, PE_FRAC, RING_MODE)
    if key not in _GRAPH_CACHE:
        _GRAPH_CACHE[key] = _build_graph(
            NI, CS, pe_frac=PE_FRAC, ring_mode=RING_MODE
        )
    nc = _GRAPH_CACHE[key]

    res = run_bass_kernel_spmd(
        nc, in_maps, core_ids=list(range(N_CORES)),
        trace=trace, **(trace_kwargs or {}),
    )
    out = np.empty((NT_TOTAL, O), np.float32)
    for c in range(N_CORES):
        oq, nh = c % O_WAY, c // O_WAY
        yc = res.results[c]["y"].reshape(N_SH, O_SH)
        out[N_SH * nh : N_SH * (nh + 1), O_SH * oq : O_SH * (oq + 1)] = yc
    return out.reshape(B, S, O), res


def kernel(x, qweight, lut, rows, cols, vals, bias):
    out, _ = _run(dict(x=x, qweight=qweight, lut=lut, rows=rows,
                       cols=cols, vals=vals, bias=bias))
    return out



# revision 3
# speedup vs baseline: 2.0218x; 2.0218x over previous
"""AnyPrecisionLinear (4-bit LUT dequant + CSR outliers + bias) on 8 TRN2 cores.

Sharding: 4-way over out_features (O) x 2-way over tokens (B*S).
Core c handles o in [1024*(c%4), +1024), tokens [4096*(c//4), +4096).

Device does all value math:
  - W (bf16) built from lut via GPSIMD local_scatter with host-computed slot
    tables (pure index preprocessing of qweight bits): pattern = full 16-entry
    lut repeated, scattered to every position, so W[o,i] = lut[o, idx[o,i]].
  - CSR outlier values cast + scattered on device into a small tile, added to W.
  - x is laid out i-major on host (pure transpose/reshape), converted
    f32->bf16 on ScalarE on device; no on-chip x transposes needed.
  - W transposed to i-major on the PE (identity transpose), drained by ScalarE.
  - GEMM on TensorE (bf16, f32 PSUM accum); bias added in the DVE drain.
Host does only layout/index work: sharding, transpose/reshape, bit-plane ->
index repack, sort/slot tables, CSR indptr parsing + dedup, output concat.
"""

import numpy as np
from contextlib import ExitStack

# Problem constants (hardcoded per harness contract).
B, S, I, O = 4, 2048, 4096, 4096
W_BITS = 4
NT_TOTAL = B * S          # 8192 tokens
N_CORES = 8
O_WAY, N_WAY = 4, 2       # sharding grid
O_SH = O // O_WAY         # 1024 out features per core
N_SH = NT_TOTAL // N_WAY  # 4096 tokens per core
NCHUNK = N_SH // 128      # 32 token chunks per core
OT = O_SH // 128          # 8 o-tiles of 128 rows per core
CH = 1024                 # i-chunk size for local_scatter (num_elems limit 2046)
NCH = I // CH             # 4
IC = I // 128             # 32 i-blocks for the GEMM
G = 512                   # tokens per x stage-in group
NG = N_SH // G            # 8
CPG = G // 128            # 4 token chunks per group

_GRAPH_CACHE = {}

import ml_dtypes

_EYE = np.eye(128, dtype=ml_dtypes.bfloat16)


def _host_indices(qweight):
    """bit-planes -> 4-bit index array [O, I] (uint8). Pure bit relayout."""
    shifts = np.arange(32, dtype=np.int32)
    # bits[b, o, w, s] = bit s of qweight[b, o, w]
    bits = ((qweight[:, :, :, None] >> shifts) & 1).astype(np.uint8)
    planew = (1 << (W_BITS - 1 - np.arange(W_BITS))).astype(np.uint8)
    idx = (bits * planew[:, None, None, None]).sum(axis=0, dtype=np.int32)
    return idx.reshape(O, I).astype(np.uint8)


def _scatter_tables(idx, rows, cols, vals):
    """Slot tables for the two device scatters (pure index preprocessing).

    Dequant scatter: slot 16*r+v holds the position of the r-th occurrence of
    lut-code v within the chunk, so scattering the repeated 16-entry lut
    pattern writes lut[o, idx[o,i]] at every position. CSR scatter: slot j of
    chunk ch holds the position of the j-th outlier; its (deduped) value is
    scattered into a zeroed tile and added on top.

    Returns:
      tbl    [O, NCH, NI]  int16  dequant scatter indices (-1 pad)
      ctb    [O, NCH, CSE] int16  csr scatter indices (-1 pad)
      cvals  [O, NCH, CSE] f32    csr value of each slot (0 pad)
      NI, CSE
    """
    # ---- CSR -> dedup'd COO ----
    nnz = cols.shape[0]
    row_ids = (np.searchsorted(rows, np.arange(nnz), side="right") - 1).astype(np.int64)
    key = row_ids * I + cols.astype(np.int64)
    uk, inv = np.unique(key, return_inverse=True)
    v2 = np.zeros(len(uk), np.float64)
    np.add.at(v2, inv, vals.astype(np.float64))
    r2 = uk // I
    c2 = uk % I
    ch2 = c2 // CH
    cl2 = (c2 % CH).astype(np.int16)
    grp = r2 * NCH + ch2  # ascending (uk sorted)
    _, gstart, gcount = np.unique(grp, return_index=True, return_counts=True)
    CSE = max(int(gcount.max()), 2)
    CSE += CSE % 2
    rank = np.arange(len(uk)) - np.repeat(gstart, gcount)
    ctb = np.full((O, NCH, CSE), -1, np.int16)
    ctb[r2, ch2, rank] = cl2
    cvals = np.zeros((O, NCH, CSE), np.float32)
    cvals[r2, ch2, rank] = v2.astype(np.float32)

    # ---- dequant slots (all 16 codes, csr positions included) ----
    idx4 = idx.reshape(O, NCH, CH).astype(np.int16)
    order = np.argsort(idx4, axis=-1, kind="stable").astype(np.int16)
    sortedv = np.take_along_axis(idx4, order.astype(np.int64), axis=-1)
    cnt = np.zeros((O, NCH, 16), np.int32)
    for v in range(16):
        cnt[:, :, v] = (idx4 == v).sum(-1)
    S = int(cnt.max())
    NI = 16 * S
    cstart = np.concatenate(
        [np.zeros((O, NCH, 1), np.int32), np.cumsum(cnt, -1)[:, :, :-1]], -1
    )
    srank = np.arange(CH)[None, None, :] - np.take_along_axis(
        cstart, sortedv.astype(np.int64), axis=-1
    )
    tbl = np.full((O, NCH, NI), -1, np.int16)
    flat = (16 * srank + sortedv).astype(np.int64)
    np.put_along_axis(tbl, flat, order, axis=-1)
    return tbl, ctb, cvals, NI, CSE


def _build_graph(NI, CS):
    import concourse.bass as bass
    import concourse.bacc as bacc
    import concourse.tile as tile
    from concourse import mybir

    dt = mybir.dt
    nc = bacc.Bacc("TRN2", target_bir_lowering=False, debug=False)

    CSE = CS
    xt_d = nc.dram_tensor("xt", [IC, 128, N_SH], dt.float32, kind="ExternalInput")
    lut_d = nc.dram_tensor("lut", [OT, 128, 16], dt.float32, kind="ExternalInput")
    qid_d = nc.dram_tensor("qidx", [OT, 128, NCH * NI], dt.int16, kind="ExternalInput")
    ctb_d = nc.dram_tensor("ctb", [OT, 128, NCH * CSE], dt.int16, kind="ExternalInput")
    cva_d = nc.dram_tensor("cvals", [OT, 128, NCH * CSE], dt.float32, kind="ExternalInput")
    bias_d = nc.dram_tensor("bias", [1, O_SH], dt.float32, kind="ExternalInput")
    eye_d = nc.dram_tensor("eye", [128, 128], dt.bfloat16, kind="ExternalInput")
    y_d = nc.dram_tensor("y", [NCHUNK, 128, O_SH], dt.float32, kind="ExternalOutput")

    with tile.TileContext(nc) as tc, ExitStack() as ctx:
        const = ctx.enter_context(tc.tile_pool(name="const", bufs=1))
        wpool = ctx.enter_context(tc.tile_pool(name="w", bufs=2))
        spool = ctx.enter_context(tc.tile_pool(name="scat", bufs=2))
        qpool = ctx.enter_context(tc.tile_pool(name="qp", bufs=4))
        xfpool = ctx.enter_context(tc.tile_pool(name="xf", bufs=6))
        xgpool = ctx.enter_context(tc.tile_pool(name="xg", bufs=2))
        yopool = ctx.enter_context(tc.tile_pool(name="yo", bufs=4))
        psum = ctx.enter_context(
            tc.tile_pool(name="ps", bufs=4, space=bass.MemorySpace.PSUM)
        )
        pst = ctx.enter_context(
            tc.tile_pool(name="pst", bufs=2, space=bass.MemorySpace.PSUM)
        )
        psb = ctx.enter_context(
            tc.tile_pool(name="psb", bufs=1, space=bass.MemorySpace.PSUM)
        )

        # Resident transposed weights: WT[p, 1024*ic + ol] = W[ol, 128*ic + p]
        WT = const.tile([128, IC * O_SH], dt.bfloat16)

        eye = const.tile([128, 128], dt.bfloat16)
        nc.scalar.dma_start(eye[:, :], eye_d[:, :])
        ones = const.tile([1, 128], dt.bfloat16)
        nc.vector.memset(ones[:, :], 1.0)

        # bias broadcast to all 128 token partitions, once
        browf = const.tile([1, O_SH], dt.float32)
        nc.scalar.dma_start(browf[:, :], bias_d[:, :])
        brow = const.tile([1, O_SH], dt.bfloat16)
        nc.vector.tensor_copy(brow[:, :], browf[:, :])
        bias128 = const.tile([128, O_SH], dt.float32)
        for blk in range(O_SH // 512):
            pb = psb.tile([128, 512], dt.float32, tag="pb")
            nc.tensor.matmul(
                pb[:, :], ones[:, :], brow[:, 512 * blk : 512 * (blk + 1)],
                start=True, stop=True,
            )
            nc.scalar.copy(bias128[:, 512 * blk : 512 * (blk + 1)], pb[:, :])

        # ---- weight build: dequant scatter + CSR scatter-add, then PE transpose
        for t in range(OT):
            lutf = spool.tile([128, 16], dt.float32, tag="lutf")
            nc.scalar.dma_start(lutf[:, :], lut_d[t])
            lutb = spool.tile([128, 16], dt.bfloat16, tag="lutb")
            nc.vector.tensor_copy(lutb[:, :], lutf[:, :])
            # full-lut pattern repeated S times (log-doubling copies)
            pat = spool.tile([128, NI], dt.bfloat16, tag="pat")
            nc.vector.tensor_copy(pat[:, 0:16], lutb[:, :])
            sz = 16
            while sz < NI:
                cp = min(sz, NI - sz)
                nc.vector.tensor_copy(pat[:, sz : sz + cp], pat[:, 0:cp])
                sz += cp
            # csr tables
            ctb = spool.tile([128, NCH * CSE], dt.int16, tag="ctb")
            nc.scalar.dma_start(ctb[:, :], ctb_d[t])
            cvf = spool.tile([128, NCH * CSE], dt.float32, tag="cvf")
            nc.scalar.dma_start(cvf[:, :], cva_d[t])
            cvb = spool.tile([128, NCH * CSE], dt.bfloat16, tag="cvb")
            nc.vector.tensor_copy(cvb[:, :], cvf[:, :])
            Wt = wpool.tile([128, I], dt.bfloat16, tag="W")
            for ch in range(NCH):
                sl = slice(ch * CH, (ch + 1) * CH)
                qix = qpool.tile([128, NI], dt.int16, tag="qix")
                nc.scalar.dma_start(qix[:, :], qid_d[t][:, ch * NI : (ch + 1) * NI])
                nc.gpsimd.local_scatter(
                    Wt[:, sl], pat[:, :], qix[:, :],
                    channels=128, num_elems=CH, num_idxs=NI,
                )
                Ct = spool.tile([128, CH], dt.bfloat16, tag="Ct")
                nc.gpsimd.local_scatter(
                    Ct[:, :], cvb[:, ch * CSE : (ch + 1) * CSE],
                    ctb[:, ch * CSE : (ch + 1) * CSE],
                    channels=128, num_elems=CH, num_idxs=CSE,
                )
                nc.vector.tensor_add(Wt[:, sl], Wt[:, sl], Ct[:, :])
                for k in range(CH // 128):
                    ic = ch * (CH // 128) + k
                    pt = pst.tile([128, 128], dt.bfloat16, tag="pt")
                    nc.tensor.transpose(
                        pt[:, :], Wt[:, 128 * ic : 128 * (ic + 1)], eye[:, :]
                    )
                    nc.scalar.copy(
                        WT[:, O_SH * ic + 128 * t : O_SH * ic + 128 * (t + 1)],
                        pt[:, :],
                    )

        # ---- GEMM: stream host-transposed x in G-token groups ----
        for g in range(NG):
            xg = []
            for ic in range(IC):
                xf = xfpool.tile([128, G], dt.float32, tag="xf")
                nc.sync.dma_start(xf[:, :], xt_d[ic][:, G * g : G * (g + 1)])
                xb = xgpool.tile([128, G], dt.bfloat16, tag=f"xg{ic}")
                nc.scalar.copy(xb[:, :], xf[:, :])
                xg.append(xb)
            for nloc in range(CPG):
                n = g * CPG + nloc
                ps0 = psum.tile([128, 512], dt.float32, tag="ps")
                ps1 = psum.tile([128, 512], dt.float32, tag="ps")
                pss = [ps0, ps1]
                for ic in range(IC):
                    lhs = xg[ic][:, 128 * nloc : 128 * (nloc + 1)]
                    for blk in range(2):
                        nc.tensor.matmul(
                            pss[blk][:, :],
                            lhs,
                            WT[:, O_SH * ic + 512 * blk : O_SH * ic + 512 * (blk + 1)],
                            start=(ic == 0), stop=(ic == IC - 1),
                        )
                for blk in range(2):
                    yo = yopool.tile([128, 512], dt.float32, tag="yo")
                    nc.vector.tensor_add(
                        yo[:, :], pss[blk][:, :],
                        bias128[:, 512 * blk : 512 * (blk + 1)],
                    )
                    nc.sync.dma_start(y_d[n][:, 512 * blk : 512 * (blk + 1)], yo[:, :])

    nc.compile()
    return nc


def _prep_inputs(x, qweight, lut, rows, cols, vals, bias):
    x = np.ascontiguousarray(np.asarray(x, dtype=np.float32))
    qweight = np.asarray(qweight, dtype=np.int32)
    lut = np.asarray(lut, dtype=np.float32)
    rows = np.asarray(rows, dtype=np.int64)
    cols = np.asarray(cols, dtype=np.int64)
    vals = np.asarray(vals, dtype=np.float32)
    bias = np.asarray(bias, dtype=np.float32)

    idx = _host_indices(qweight)
    tbl, ctb, cvals, NI, CSE = _scatter_tables(idx, rows, cols, vals)

    x2 = x.reshape(NT_TOTAL, I)
    # i-major x per token shard (pure layout), shared by the 4 o-shard cores
    xts = []
    for nh in range(N_WAY):
        nsl = slice(N_SH * nh, N_SH * (nh + 1))
        xts.append(np.ascontiguousarray(x2[nsl].T).reshape(IC, 128, N_SH))
    in_maps = []
    for c in range(N_CORES):
        oq, nh = c % O_WAY, c // O_WAY
        osl = slice(O_SH * oq, O_SH * (oq + 1))
        in_maps.append(
            {
                "xt": xts[nh],
                "lut": np.ascontiguousarray(lut[osl].reshape(OT, 128, 16)),
                # chunk-major per o-tile row: [OT, 128, NCH*NI]
                "qidx": np.ascontiguousarray(tbl[osl].reshape(OT, 128, NCH * NI)),
                "ctb": np.ascontiguousarray(ctb[osl].reshape(OT, 128, NCH * CSE)),
                "cvals": np.ascontiguousarray(
                    cvals[osl].reshape(OT, 128, NCH * CSE)
                ),
                "bias": np.ascontiguousarray(bias[osl].reshape(1, O_SH)),
                "eye": _EYE,
            }
        )
    return in_maps, NI, CSE


def _run(inputs, trace=False, trace_kwargs=None):
    from concourse.bass_utils import run_bass_kernel_spmd

    in_maps, NI, CS = _prep_inputs(**inputs)

    key = (NI, CS)
    if key not in _GRAPH_CACHE:
        _GRAPH_CACHE[key] = _build_graph(NI, CS)
    nc = _GRAPH_CACHE[key]

    res = run_bass_kernel_spmd(
        nc, in_maps, core_ids=list(range(N_CORES)),
        trace=trace, **(trace_kwargs or {}),
    )
    out = np.empty((NT_TOTAL, O), np.float32)
    for c in range(N_CORES):
        oq, nh = c % O_WAY, c // O_WAY
        yc = res.results[c]["y"].reshape(N_SH, O_SH)
        out[N_SH * nh : N_SH * (nh + 1), O_SH * oq : O_SH * (oq + 1)] = yc
    return out.reshape(B, S, O), res


def kernel(x, qweight, lut, rows, cols, vals, bias):
    out, _ = _run(dict(x=x, qweight=qweight, lut=lut, rows=rows,
                       cols=cols, vals=vals, bias=bias))
    return out
